# revision 16
# baseline (speedup 1.0000x reference)
"""Trainium2 Bass kernel for the LNN Euler-Lagrange residual.

Math: for a ReLU MLP Lagrangian L(q, qdot) the JAX second-derivative term
d/dt(dL/dqdot) is identically zero (piecewise-linear network), so the
reference output reduces to -dL/dq:

    z1 = x @ W1 + b1          s1 = z1 > 0
    z2 = a1 @ W2 + b2         s2 = z2 > 0      a1 = relu(z1)
    pre1 = s2 @ W2T_eff       (W2T_eff[j,i] = w3[j] * W2[i,j])
    out  = (pre1 * s1) @ (-W1[:32,:].T)

Layout: feature-major (features on partitions, batch streams as matmul
free dim). Host pre-transposes the input shard to [128, B_core/2] fp16.
Two batch groups are packed on the 128 partitions via host-built 128x128
block stationary matrices, so every matmul uses the full PE array K=128.

All matmuls run in fp16 (1 cyc/row on the PE): z1/z2 get ~3e-4 relative
accuracy which keeps ReLU-mask flips vs the fp32 reference to ~0.03% of
elements; the mask/value path (L3/L4) runs in bf16.

Pipeline is pair-based (pair = 2 chunks of 512 cols): elementwise ops and
DMAs run once per pair at [128,1024] to amortize fixed costs. L4 packs the
even chunk's [64,512] output into PSUM partitions 0:64 and the odd chunk's
into 64:128 so a single [128,512] fp32 store per pair goes straight from
PSUM to DRAM.

Engine assignment per pair: ACT relu (1 op), DVE s1-mask (fp16 4x mode) +
t1 mult, Pool s2-mask, PE 8 matmuls, SP ring input DMA, ACT ring output
DMA.
"""

import sys

sys.path.insert(0, "/opt/trn_rl_repo")

from contextlib import ExitStack

import numpy as np

B, D, H = 262144, 32, 64
NCORES = 8
BC = B // NCORES          # samples per core
G = BC // 2               # samples per group (2 groups packed on partitions)
CHUNK = 512               # matmul free-dim (one fp32 PSUM bank)
PAIR = 2 * CHUNK          # elementwise/DMA granularity
NPAIRS = G // PAIR

_CACHE = {}


def _build(bc=BC):
    import concourse.bass as bass
    import concourse.tile as tile
    from concourse import bacc, mybir

    f32 = mybir.dt.float32
    f16 = mybir.dt.float16
    bf16 = mybir.dt.bfloat16
    Relu = mybir.ActivationFunctionType.Relu
    Copy = mybir.ActivationFunctionType.Copy
    is_gt = mybir.AluOpType.is_gt
    mult = mybir.AluOpType.mult

    g = bc // 2
    npairs = g // PAIR

    nc = bacc.Bacc("TRN2", target_bir_lowering=False, debug=False)

    # xT rows: p = grp*64 + f (group grp's feature f); cols: sample in group
    xT = nc.dram_tensor("xT", [128, g], f16, kind="ExternalInput").ap()
    S1 = nc.dram_tensor("S1", [128, 128], f16, kind="ExternalInput").ap()
    S2 = nc.dram_tensor("S2", [128, 128], f16, kind="ExternalInput").ap()
    S3 = nc.dram_tensor("S3", [128, 128], bf16, kind="ExternalInput").ap()
    S4 = nc.dram_tensor("S4", [128, 64], bf16, kind="ExternalInput").ap()
    BIASES = nc.dram_tensor("BIASES", [128, 2], f32, kind="ExternalInput").ap()
    # outT rows (per pair column block): 0:32 A-even, 32:64 B-even,
    # 64:96 A-odd, 96:128 B-odd outputs
    outT = nc.dram_tensor("outT", [128, g // 2], bf16,
                          kind="ExternalOutput").ap()

    SPL = 384  # ACT/Pool split point of the out-copy

    with tile.TileContext(nc) as tc, ExitStack() as ctx:
        wp = ctx.enter_context(tc.tile_pool(name="w", bufs=1))
        xs_p = ctx.enter_context(tc.tile_pool(name="xs", bufs=3))
        a1_p = ctx.enter_context(tc.tile_pool(name="a1", bufs=3))
        s1_p = ctx.enter_context(tc.tile_pool(name="s1", bufs=3))
        s2_p = ctx.enter_context(tc.tile_pool(name="s2", bufs=4))
        t1_p = ctx.enter_context(tc.tile_pool(name="t1", bufs=4))
        ot_p = ctx.enter_context(tc.tile_pool(name="ot", bufs=3))
        # PSUM: 4 tensors x 2 rotating [128,512] chunk tiles = 8 banks
        z1_p = ctx.enter_context(tc.tile_pool(name="z1", bufs=2, space="PSUM"))
        z2_p = ctx.enter_context(tc.tile_pool(name="z2", bufs=2, space="PSUM"))
        p1_p = ctx.enter_context(tc.tile_pool(name="p1", bufs=2, space="PSUM"))
        op_p = ctx.enter_context(tc.tile_pool(name="op", bufs=2, space="PSUM"))

        s1w = wp.tile([128, 128], f16, tag="s1w")
        s2w = wp.tile([128, 128], f16, tag="s2w")
        s3w = wp.tile([128, 128], bf16, tag="s3w")
        s4w = wp.tile([128, 64], bf16, tag="s4w")
        bia = wp.tile([128, 2], f32, tag="bia")
        nc.sync.dma_start(out=s1w[:], in_=S1)
        nc.sync.dma_start(out=s2w[:], in_=S2)
        nc.sync.dma_start(out=s3w[:], in_=S3)
        nc.sync.dma_start(out=s4w[:], in_=S4)
        nc.sync.dma_start(out=bia[:], in_=BIASES)

        # Rolling modulo schedule over pairs. At step s the PE stream runs
        # L1(s), L2(s-1), L3(s-2), L4(s-3) so each cross-engine elementwise
        # stage gets ~6 matmuls of PE time to complete without stalling it.
        # All PSUM tensors are double-buffered per chunk so buffer-reuse
        # waits land one full pair behind the producer.
        xs_t = {}
        a1_t = {}
        s1_t = {}
        z1_t = {}
        z2_t = {}
        s2_t = {}
        p1_t = {}
        t1_t = {}
        op_t = {}
        for s in range(npairs + 4):
            if s < npairs:
                # input DMA for pair s (SP ring)
                xs = xs_p.tile([128, PAIR], f16, tag="xs")
                nc.sync.dma_start(out=xs[:], in_=xT[:, s * PAIR:(s + 1) * PAIR])
                xs_t[s] = xs

            if s < npairs:
                # L1: z1 = S1.T @ xs (fp16; A->p0:64, B->p64:128)
                # a1 = relu(z1 + b1) -> fp16 (ACT) per chunk
                a1 = a1_p.tile([128, PAIR], f16, tag="a1")
                zs = []
                for h in range(2):
                    z1p = z1_p.tile([128, CHUNK], f32, tag="z1")
                    nc.tensor.matmul(
                        z1p[:], lhsT=s1w[:],
                        rhs=xs_t[s][:, bass.ts(h, CHUNK)],
                        start=True, stop=True)
                    nc.scalar.activation(out=a1[:, bass.ts(h, CHUNK)],
                                         in_=z1p[:], func=Relu,
                                         bias=bia[:, 0:1], scale=1.0)
                    zs.append(z1p)
                z1_t[s] = zs
                a1_t[s] = a1
                # s1 = a1 > 0 (DVE, fp16 all-SBUF 4x mode, whole pair)
                s1m = s1_p.tile([128, PAIR], f16, tag="s1")
                nc.vector.tensor_scalar(out=s1m[:], in0=a1[:], scalar1=0.0,
                                        scalar2=None, op0=is_gt)
                s1_t[s] = s1m
                del xs_t[s]

            if 0 <= s - 1 < npairs:
                i = s - 1
                # L2: z2 = S2.T @ a1 (fp16, groups swap halves)
                # s2 = (z2 > -b2) bf16 (Pool) per chunk
                zs = []
                ss = []
                for h in range(2):
                    z2p = z2_p.tile([128, CHUNK], f32, tag="z2")
                    nc.tensor.matmul(
                        z2p[:], lhsT=s2w[:],
                        rhs=a1_t[i][:, bass.ts(h, CHUNK)],
                        start=True, stop=True)
                    s2m = s2_p.tile([128, CHUNK], bf16, tag="s2")
                    nc.gpsimd.tensor_scalar(out=s2m[:], in0=z2p[:],
                                            scalar1=bia[:, 1:2],
                                            scalar2=None, op0=is_gt)
                    zs.append(z2p)
                    ss.append(s2m)
                z2_t[i] = zs
                s2_t[i] = ss
                del a1_t[i], z1_t[i]

            if 0 <= s - 2 < npairs:
                i = s - 2
                # L3: pre1 = S3.T @ s2 (bf16); t1 = pre1 * s1 (DVE) per chunk
                ps = []
                ts = []
                for h in range(2):
                    p1p = p1_p.tile([128, CHUNK], f32, tag="p1")
                    nc.tensor.matmul(
                        p1p[:], lhsT=s3w[:], rhs=s2_t[i][h][:],
                        start=True, stop=True)
                    t1 = t1_p.tile([128, CHUNK], bf16, tag="t1")
                    nc.vector.tensor_tensor(
                        out=t1[:], in0=p1p[:],
                        in1=s1_t[i][:, bass.ts(h, CHUNK)], op=mult)
                    ps.append(p1p)
                    ts.append(t1)
                p1_t[i] = ps
                t1_t[i] = ts
                del s2_t[i], z2_t[i], s1_t[i]

            if 0 <= s - 3 < npairs:
                i = s - 3
                # L4: even chunk -> op[0:64], odd chunk -> op[64:128]
                op_ = op_p.tile([128, CHUNK], f32, tag="op")
                nc.tensor.matmul(
                    op_[0:64, :], lhsT=s4w[:],
                    rhs=t1_t[i][0][:], start=True, stop=True)
                nc.tensor.matmul(
                    op_[64:128, :], lhsT=s4w[:],
                    rhs=t1_t[i][1][:], start=True, stop=True)
                op_t[i] = op_
                # PSUM f32 -> SBUF bf16: split ACT/Pool, store on ACT ring
                ot = ot_p.tile([128, CHUNK], bf16, tag="ot")
                nc.scalar.activation(out=ot[:, 0:SPL], in_=op_[:, 0:SPL],
                                     func=Copy)
                nc.gpsimd.tensor_copy(out=ot[:, SPL:CHUNK],
                                      in_=op_[:, SPL:CHUNK])
                nc.scalar.dma_start(
                    out=outT[:, i * CHUNK:(i + 1) * CHUNK], in_=ot[:])
                del t1_t[i], p1_t[i]

    nc.compile()
    return nc


def _get_nc(bc=BC):
    if bc not in _CACHE:
        _CACHE[bc] = _build(bc)
    return _CACHE[bc]


def _host_prep(W1, b1, W2, b2, W3, b3):
    import ml_dtypes

    w3 = np.asarray(W3)[:, 0].astype(np.float32)
    W1 = np.asarray(W1, np.float32)
    W2 = np.asarray(W2, np.float32)
    b1 = np.asarray(b1, np.float32)
    b2 = np.asarray(b2, np.float32)

    S1 = np.zeros((128, 128), np.float32)
    S1[:64, :64] = W1
    S1[64:, 64:] = W1
    S2 = np.zeros((128, 128), np.float32)
    S2[:64, 64:] = W2
    S2[64:, :64] = W2
    S3s = (W2 * w3[None, :]).T  # [j, i] = w3[j] * W2[i, j]
    S3 = np.zeros((128, 128), np.float32)
    S3[64:, :64] = S3s  # A: s2 at p64:128 -> pre1 at p0:64
    S3[:64, 64:] = S3s  # B: s2 at p0:64   -> pre1 at p64:128
    S4s = -(W1[:32, :].T)  # [64, 32]
    S4 = np.zeros((128, 64), np.float32)
    S4[:64, :32] = S4s   # A: t1 p0:64   -> out p0:32
    S4[64:, 32:] = S4s   # B: t1 p64:128 -> out p32:64
    BIASES = np.zeros((128, 2), np.float32)
    BIASES[:, 0] = np.concatenate([b1, b1])
    BIASES[:, 1] = -np.concatenate([b2, b2])
    return {
        "S1": S1.astype(np.float16),
        "S2": S2.astype(np.float16),
        "S3": S3.astype(ml_dtypes.bfloat16),
        "S4": S4.astype(ml_dtypes.bfloat16),
        "BIASES": BIASES,
    }


def kernel(inputs, W1, b1, W2, b2, W3, b3):
    from concourse.bass_utils import run_bass_kernel_spmd

    x = np.asarray(inputs, np.float32)
    consts = _host_prep(W1, b1, W2, b2, W3, b3)

    in_maps = []
    for k in range(NCORES):
        xc = x[k * BC:(k + 1) * BC].astype(np.float16)   # [BC, 64]
        # rows p = grp*64 + f: group A samples [0,G) then group B [G,2G)
        xTk = np.ascontiguousarray(
            np.concatenate([xc[:G].T, xc[G:].T], axis=0))  # [128, G] fp16
        in_maps.append({"xT": xTk, **consts})

    nc = _get_nc()
    res = run_bass_kernel_spmd(nc, in_maps, core_ids=list(range(NCORES)),
                               trace=False)
    outs = []
    for k in range(NCORES):
        oT = np.asarray(res.results[k]["outT"], np.float32)  # [128, G//2]
        # rows: eo*64 + grp*32 + f ; cols: pair*512 + j
        v = oT.reshape(2, 2, 32, NPAIRS, CHUNK)
        # -> [grp, pair, eo, j, f] -> [grp, G, 32]
        w = np.transpose(v, (1, 3, 0, 4, 2)).reshape(2, G, 32)
        outs.append(w[0])
        outs.append(w[1])
    out = np.concatenate(outs, axis=0).astype(np.float32)
    kernel._last_result = res
    return out


# revision 24
# speedup vs baseline: 1.0923x; 1.0923x over previous
"""Trainium2 Bass kernel for the LNN Euler-Lagrange residual.

Math: for a ReLU MLP Lagrangian L(q, qdot) the JAX second-derivative term
d/dt(dL/dqdot) is identically zero (piecewise-linear network), so the
reference output reduces to -dL/dq:

    z1 = x @ W1 + b1          s1 = z1 > 0
    z2 = a1 @ W2 + b2         s2 = z2 > 0      a1 = relu(z1)
    pre1 = s2 @ W2T_eff       (W2T_eff[j,i] = w3[j] * W2[i,j])
    out  = (pre1 * s1) @ (-W1[:32,:].T)

Layout: feature-major (features on partitions, batch streams as matmul
free dim). Host pre-transposes the input shard to [128, B_core/2] fp16.
Two batch groups are packed on the 128 partitions via host-built 128x128
block stationary matrices, so every matmul uses the full PE array K=128.

All matmuls run in fp16 (1 cyc/row on the PE): z1/z2 get ~3e-4 relative
accuracy which keeps ReLU-mask flips vs the fp32 reference to ~0.03% of
elements; the mask/value path (L3/L4) runs in bf16.

Pipeline is pair-based (pair = 2 chunks of 512 cols): elementwise ops and
DMAs run once per pair at [128,1024] to amortize fixed costs. L4 packs the
even chunk's [64,512] output into PSUM partitions 0:64 and the odd chunk's
into 64:128 so a single [128,512] fp32 store per pair goes straight from
PSUM to DRAM.

Engine assignment per pair: ACT relu (1 op), DVE s1-mask (fp16 4x mode) +
t1 mult, Pool s2-mask, PE 8 matmuls, SP ring input DMA, ACT ring output
DMA.
"""

import sys

sys.path.insert(0, "/opt/trn_rl_repo")

from contextlib import ExitStack

import numpy as np

B, D, H = 262144, 32, 64
NCORES = 8
BC = B // NCORES          # samples per core
G = BC // 2               # samples per group (2 groups packed on partitions)
CHUNK = 512               # matmul free-dim (one fp32 PSUM bank)
PAIR = 2 * CHUNK          # elementwise/DMA granularity
NPAIRS = G // PAIR

_CACHE = {}


def _build(bc=BC):
    import concourse.bass as bass
    import concourse.tile as tile
    from concourse import bacc, mybir

    f32 = mybir.dt.float32
    f16 = mybir.dt.float16
    bf16 = mybir.dt.bfloat16
    Relu = mybir.ActivationFunctionType.Relu
    Copy = mybir.ActivationFunctionType.Copy
    is_gt = mybir.AluOpType.is_gt
    mult = mybir.AluOpType.mult

    g = bc // 2
    npairs = g // PAIR

    nc = bacc.Bacc("TRN2", target_bir_lowering=False, debug=False)

    # xT rows: p = grp*64 + f (group grp's feature f); cols: sample in group
    xT = nc.dram_tensor("xT", [128, g], f16, kind="ExternalInput").ap()
    # All six stationary/bias tensors packed into one byte blob so the
    # prologue is a single DMA: S1 f16 [128,128] | S2 f16 [128,128] |
    # S3 bf16 [128,128] | S4 bf16 [128,64] | BIASES f32 [128,2]
    CONSTS = nc.dram_tensor("CONSTS", [128, 904], mybir.dt.uint8,
                            kind="ExternalInput").ap()
    # outT rows (per pair column block): 0:32 A-even, 32:64 B-even,
    # 64:96 A-odd, 96:128 B-odd outputs
    outT = nc.dram_tensor("outT", [128, g // 2], bf16,
                          kind="ExternalOutput").ap()

    with tile.TileContext(nc) as tc, ExitStack() as ctx:
        wp = ctx.enter_context(tc.tile_pool(name="w", bufs=1))
        xs_p = ctx.enter_context(tc.tile_pool(name="xs", bufs=3))
        a1_p = ctx.enter_context(tc.tile_pool(name="a1", bufs=3))
        s1_p = ctx.enter_context(tc.tile_pool(name="s1", bufs=3))
        s2_p = ctx.enter_context(tc.tile_pool(name="s2", bufs=4))
        t1_p = ctx.enter_context(tc.tile_pool(name="t1", bufs=4))
        ot_p = ctx.enter_context(tc.tile_pool(name="ot", bufs=3))
        # PSUM: 4 tensors x 2 rotating [128,512] chunk tiles = 8 banks
        z1_p = ctx.enter_context(tc.tile_pool(name="z1", bufs=2, space="PSUM"))
        z2_p = ctx.enter_context(tc.tile_pool(name="z2", bufs=2, space="PSUM"))
        p1_p = ctx.enter_context(tc.tile_pool(name="p1", bufs=2, space="PSUM"))
        op_p = ctx.enter_context(tc.tile_pool(name="op", bufs=2, space="PSUM"))

        cw = wp.tile([128, 904], mybir.dt.uint8, tag="cw")
        nc.sync.dma_start(out=cw[:], in_=CONSTS)
        s1w = cw[:, 0:256].bitcast(f16)
        s2w = cw[:, 256:512].bitcast(f16)
        s3w = cw[:, 512:768].bitcast(bf16)
        s4w = cw[:, 768:896].bitcast(bf16)
        bia = cw[:, 896:904].bitcast(f32)

        # Rolling modulo schedule over pairs. Per step s the PE stream runs
        # L1(s), L2(s-1), L3(s-2), L4(s-3); elementwise stages run one step
        # after their producer so no engine op waits on a result computed
        # late in the same step. Engine issue order avoids in-order
        # head-of-line coupling: ACT does copy(s-3) BEFORE relu(s); DVE does
        # t1(s-2) before s1(s-1). All PSUM tensors are double-buffered per
        # chunk so buffer-reuse waits land one full pair behind the producer.
        xs_t = {}
        a1_t = {}
        s1_t = {}
        z1_t = {}
        z2_t = {}
        s2_t = {}
        p1_t = {}
        t1_t = {}
        op_t = {}
        ot_t = {}
        for s in range(npairs + 5):
            if s == 0:
                for j in range(min(2, npairs)):
                    xs = xs_p.tile([128, PAIR], f16, tag="xs")
                    nc.sync.dma_start(
                        out=xs[:], in_=xT[:, j * PAIR:(j + 1) * PAIR])
                    xs_t[j] = xs
            if s + 2 < npairs:
                # prefetch input pair s+2 (SP ring)
                xs = xs_p.tile([128, PAIR], f16, tag="xs")
                nc.sync.dma_start(
                    out=xs[:], in_=xT[:, (s + 2) * PAIR:(s + 3) * PAIR])
                xs_t[s + 2] = xs

            if 0 <= s - 4 < npairs:
                i = s - 4
                # out-copy of pair i (L4(i) ran last step): PSUM -> SBUF bf16
                # issued FIRST on ACT so relu(s) queues behind it, not the
                # reverse; then the store on the ACT HWDGE ring.
                ot = ot_p.tile([128, CHUNK], bf16, tag="ot")
                nc.scalar.activation(out=ot[:], in_=op_t[i][:], func=Copy)
                nc.scalar.dma_start(
                    out=outT[:, i * CHUNK:(i + 1) * CHUNK], in_=ot[:])
                ot_t[i] = ot
                del op_t[i]

            if s < npairs:
                # L1: z1 = S1.T @ xs (fp16; A->p0:64, B->p64:128)
                # a1 = relu(z1 + b1) -> fp16 (ACT) per chunk
                a1 = a1_p.tile([128, PAIR], f16, tag="a1")
                zs = []
                for h in range(2):
                    z1p = z1_p.tile([128, CHUNK], f32, tag="z1")
                    nc.tensor.matmul(
                        z1p[:], lhsT=s1w,
                        rhs=xs_t[s][:, bass.ts(h, CHUNK)],
                        start=True, stop=True)
                    nc.scalar.activation(out=a1[:, bass.ts(h, CHUNK)],
                                         in_=z1p[:], func=Relu,
                                         bias=bia[:, 0:1], scale=1.0)
                    zs.append(z1p)
                z1_t[s] = zs
                a1_t[s] = a1

            if 0 <= s - 1 < npairs:
                i = s - 1
                # L2: z2 = S2.T @ a1 (fp16, groups swap halves)
                # s2 = (z2 > -b2) bf16 (Pool) per chunk
                zs = []
                ss = []
                for h in range(2):
                    z2p = z2_p.tile([128, CHUNK], f32, tag="z2")
                    nc.tensor.matmul(
                        z2p[:], lhsT=s2w,
                        rhs=a1_t[i][:, bass.ts(h, CHUNK)],
                        start=True, stop=True)
                    s2m = s2_p.tile([128, CHUNK], bf16, tag="s2")
                    nc.gpsimd.tensor_scalar(out=s2m[:], in0=z2p[:],
                                            scalar1=bia[:, 1:2],
                                            scalar2=None, op0=is_gt)
                    zs.append(z2p)
                    ss.append(s2m)
                z2_t[i] = zs
                s2_t[i] = ss

            if 0 <= s - 2 < npairs:
                i = s - 2
                # L3: pre1 = S3.T @ s2 (bf16); t1 = pre1 * s1 (DVE) per chunk
                ps = []
                ts = []
                for h in range(2):
                    p1p = p1_p.tile([128, CHUNK], f32, tag="p1")
                    nc.tensor.matmul(
                        p1p[:], lhsT=s3w, rhs=s2_t[i][h][:],
                        start=True, stop=True)
                    t1 = t1_p.tile([128, CHUNK], bf16, tag="t1")
                    nc.vector.tensor_tensor(
                        out=t1[:], in0=p1p[:],
                        in1=s1_t[i][:, bass.ts(h, CHUNK)], op=mult)
                    ps.append(p1p)
                    ts.append(t1)
                p1_t[i] = ps
                t1_t[i] = ts
                del s2_t[i], z2_t[i], s1_t[i]

            if 0 <= s - 1 < npairs:
                i = s - 1
                # s1 = a1 > 0 (DVE fp16 all-SBUF fast mode, whole pair).
                # Issued AFTER t1(s-2) so the old pair's t1 isn't queued
                # behind a wait on this pair's relu.
                s1m = s1_p.tile([128, PAIR], f16, tag="s1")
                nc.vector.tensor_scalar(out=s1m[:], in0=a1_t[i][:],
                                        scalar1=0.0,
                                        scalar2=None, op0=is_gt)
                s1_t[i] = s1m
                del a1_t[i], z1_t[i], xs_t[i]

            if 0 <= s - 3 < npairs:
                i = s - 3
                # L4: even chunk -> op[0:64], odd chunk -> op[64:128]
                op_ = op_p.tile([128, CHUNK], f32, tag="op")
                nc.tensor.matmul(
                    op_[0:64, :], lhsT=s4w,
                    rhs=t1_t[i][0][:], start=True, stop=True)
                nc.tensor.matmul(
                    op_[64:128, :], lhsT=s4w,
                    rhs=t1_t[i][1][:], start=True, stop=True)
                op_t[i] = op_
                del t1_t[i], p1_t[i]

    nc.compile()
    return nc


def _get_nc(bc=BC):
    if bc not in _CACHE:
        _CACHE[bc] = _build(bc)
    return _CACHE[bc]


def _host_prep(W1, b1, W2, b2, W3, b3):
    import ml_dtypes

    w3 = np.asarray(W3)[:, 0].astype(np.float32)
    W1 = np.asarray(W1, np.float32)
    W2 = np.asarray(W2, np.float32)
    b1 = np.asarray(b1, np.float32)
    b2 = np.asarray(b2, np.float32)

    S1 = np.zeros((128, 128), np.float32)
    S1[:64, :64] = W1
    S1[64:, 64:] = W1
    S2 = np.zeros((128, 128), np.float32)
    S2[:64, 64:] = W2
    S2[64:, :64] = W2
    S3s = (W2 * w3[None, :]).T  # [j, i] = w3[j] * W2[i, j]
    S3 = np.zeros((128, 128), np.float32)
    S3[64:, :64] = S3s  # A: s2 at p64:128 -> pre1 at p0:64
    S3[:64, 64:] = S3s  # B: s2 at p0:64   -> pre1 at p64:128
    S4s = -(W1[:32, :].T)  # [64, 32]
    S4 = np.zeros((128, 64), np.float32)
    S4[:64, :32] = S4s   # A: t1 p0:64   -> out p0:32
    S4[64:, 32:] = S4s   # B: t1 p64:128 -> out p32:64
    BIASES = np.zeros((128, 2), np.float32)
    BIASES[:, 0] = np.concatenate([b1, b1])
    BIASES[:, 1] = -np.concatenate([b2, b2])
    blob = np.concatenate([
        S1.astype(np.float16).view(np.uint8),
        S2.astype(np.float16).view(np.uint8),
        S3.astype(ml_dtypes.bfloat16).view(np.uint8),
        S4.astype(ml_dtypes.bfloat16).view(np.uint8),
        BIASES.view(np.uint8),
    ], axis=1)  # [128, 904]
    return {"CONSTS": np.ascontiguousarray(blob)}


def kernel(inputs, W1, b1, W2, b2, W3, b3):
    from concourse.bass_utils import run_bass_kernel_spmd

    x = np.asarray(inputs, np.float32)
    consts = _host_prep(W1, b1, W2, b2, W3, b3)

    in_maps = []
    for k in range(NCORES):
        xc = x[k * BC:(k + 1) * BC].astype(np.float16)   # [BC, 64]
        # rows p = grp*64 + f: group A samples [0,G) then group B [G,2G)
        xTk = np.ascontiguousarray(
            np.concatenate([xc[:G].T, xc[G:].T], axis=0))  # [128, G] fp16
        in_maps.append({"xT": xTk, **consts})

    nc = _get_nc()
    res = run_bass_kernel_spmd(nc, in_maps, core_ids=list(range(NCORES)),
                               trace=False)
    outs = []
    for k in range(NCORES):
        oT = np.asarray(res.results[k]["outT"], np.float32)  # [128, G//2]
        # rows: eo*64 + grp*32 + f ; cols: pair*512 + j
        v = oT.reshape(2, 2, 32, NPAIRS, CHUNK)
        # -> [grp, pair, eo, j, f] -> [grp, G, 32]
        w = np.transpose(v, (1, 3, 0, 4, 2)).reshape(2, G, 32)
        outs.append(w[0])
        outs.append(w[1])
    out = np.concatenate(outs, axis=0).astype(np.float32)
    kernel._last_result = res
    return out
